# revision 6
# baseline (speedup 1.0000x reference)
"""Trainium2 Bass kernel for nn_Cholesky_from_z.

Reference computation (per batch sample b, n=128):
    s starts at 0 per row i; for column j: col = z[i,j]*sqrt(1-s) below diag,
    sqrt(1-s) on diag, 0 above; s += col^2.
Closed form: 1-s at (row i, col j) = prod_{k<j} (1 - z[i,k]^2), so
    L[i,j] = z[i,j] * sqrt(prod_{k<j}(1-z[i,k]^2))   (j < i)
    L[i,i] =          sqrt(prod_{k<i}(1-z[i,k]^2))
i.e. an exclusive cumulative product of (1-z^2) along each matrix row,
independent per row and per sample.

Device mapping: each sample's strictly-lower entries are packed row-major with
a 1.0 sentinel appended after each row (the "diagonal slot"), 8256 slots total.
One leading 1.0 column is prepended so every chunk can read one element back
for the shift.  On device, per [128 samples x chunk] tile:
    u = Square(z)                      (ACT)
    a = 1 - u, shifted one slot right  (DVE tensor_scalar)
        -> a = 0 exactly at each row-start slot (previous slot is the 1.0
           sentinel), which marks segment boundaries for free
    b = (a == 0) ? 1 : 0               (DVE, computed once; reused)
    d = scan: state = a*state + b      (DVE tensor_tensor_scan = segmented
                                        exclusive cumprod, carried across
                                        chunks via `initial`)
    q = Sqrt(d)                        (ACT)
    out = z * q                        (DVE)  [diag slot: 1 * q = q]
Batch dim (2048) is sharded 256 samples per core across 8 cores; each core
processes 2 partition-blocks of 128 samples.
"""

import sys

if "/opt/trn_rl_repo" not in sys.path:
    sys.path.insert(0, "/opt/trn_rl_repo")

import numpy as np

B = 2048
N = 128
NZ = N * (N - 1) // 2          # 8128 strictly-lower entries
PACKED = NZ + N                # 8256 slots incl. diagonal sentinels
NCORES = 8
B_CORE = B // NCORES           # 256
CHUNK = 2064                   # 4 chunks of 2064 = 8256
NCHUNK = PACKED // CHUNK

# --- host-side index maps ---------------------------------------------------
# packed slot order: row i -> [z[i,0..i-1], diag_i]; row-start offset i(i+1)/2
_rows, _cols = np.tril_indices(N, -1)                  # row-major strict lower
_strict_slots = (_rows * (_rows + 1) // 2 + _cols).astype(np.int64)
_diag_slots = (np.arange(N) * (np.arange(N) + 1) // 2 + np.arange(N)).astype(np.int64)
# position of each packed slot in the dense [128,128] row-major output
_out_pos = np.empty(PACKED, np.int64)
_out_pos[_strict_slots] = _rows * N + _cols
_out_pos[_diag_slots] = np.arange(N) * N + np.arange(N)

_prog_cache = {}


def _build_program():
    import concourse.bacc as bacc
    import concourse.mybir as mybir
    from concourse.tile import TileContext

    f32 = mybir.dt.float32
    Alu = mybir.AluOpType
    Act = mybir.ActivationFunctionType

    nc = bacc.Bacc("TRN2", target_bir_lowering=False, debug=False,
                   num_devices=NCORES)
    zp = nc.dram_tensor("zp", [B_CORE, PACKED + 1], f32,
                        kind="ExternalInput").ap()
    lp = nc.dram_tensor("lp", [B_CORE, PACKED], f32,
                        kind="ExternalOutput").ap()

    NBLK = B_CORE // 128
    with TileContext(nc) as tc:
        with (
            tc.tile_pool(name="io", bufs=2) as io_pool,
            tc.tile_pool(name="ul", bufs=3) as ul_pool,
            tc.tile_pool(name="wq", bufs=2) as wq_pool,
            tc.tile_pool(name="dp", bufs=2) as dpool,
            tc.tile_pool(name="bpool", bufs=1) as bpool,
        ):
            # SBUF/partition @ C=2064: zt 2t*2*8.26K=33K, u|lt 2t*3=49.6K,
            # a|q 2t*2=33K, d 2t*2=33K, b 4*8.26K=33K  -> ~182K < 192K
            btiles = []
            dprev = [None] * NBLK
            # interleave blocks within each chunk step: two independent scan
            # chains keep DVE fed while ACT/GPSIMD/DMA work on the other block
            for ch in range(NCHUNK):
                c0 = ch * CHUNK
                for blk in range(NBLK):
                    r0 = blk * 128
                    zt = io_pool.tile([128, CHUNK + 1], f32, tag=f"zt{blk}")
                    nc.sync.dma_start(out=zt[:], in_=zp[r0:r0 + 128, c0:c0 + CHUNK + 1])

                    # u dead after `a`; lt reuses its slots (tag share)
                    u = ul_pool.tile([128, CHUNK + 1], f32, tag=f"u{blk}")
                    nc.scalar.activation(u[:], zt[:], Act.Square)

                    # a = 1 - u, shifted one slot; load-balance across engines
                    a = wq_pool.tile([128, CHUNK + 1], f32, tag=f"w{blk}")
                    if (ch * NBLK + blk) % 4 == 0:
                        nc.vector.tensor_scalar(a[:, 0:CHUNK], u[:, 0:CHUNK],
                                                -1.0, 1.0, Alu.mult, Alu.add)
                    else:
                        nc.scalar.activation(a[:, 0:CHUNK], u[:, 0:CHUNK],
                                             Act.Copy, bias=1.0, scale=-1.0)

                    if blk == 0:
                        bt = bpool.tile([128, CHUNK], f32, tag=f"b{ch}")
                        nc.vector.tensor_scalar(bt[:], a[:, 0:CHUNK], 0.0, None,
                                                Alu.is_equal)
                        btiles.append(bt)
                    bt = btiles[ch]

                    d = dpool.tile([128, CHUNK], f32, tag=f"d{blk}")
                    init = 1.0 if ch == 0 else dprev[blk][:, CHUNK - 1:CHUNK]
                    nc.vector.tensor_tensor_scan(d[:], a[:, 0:CHUNK], bt[:], init,
                                                 Alu.mult, Alu.add)
                    dprev[blk] = d

                    # a dead after scan; q reuses its slots (same tag)
                    q = wq_pool.tile([128, CHUNK + 1], f32, tag=f"w{blk}")
                    nc.scalar.activation(q[:, 0:CHUNK], d[:], Act.Sqrt)

                    # u slots free again; lt shares them
                    lt = ul_pool.tile([128, CHUNK + 1], f32, tag=f"u{blk}")
                    nc.gpsimd.tensor_mul(lt[:, 0:CHUNK], zt[:, 1:CHUNK + 1],
                                         q[:, 0:CHUNK])
                    nc.sync.dma_start(out=lp[r0:r0 + 128, c0:c0 + CHUNK],
                                      in_=lt[:, 0:CHUNK])
    nc.compile()
    return nc


def _get_program():
    if "nc" not in _prog_cache:
        _prog_cache["nc"] = _build_program()
    return _prog_cache["nc"]


def _run(in_maps, **kw):
    from concourse.bass_utils import run_bass_kernel_spmd

    nc = _get_program()
    return run_bass_kernel_spmd(nc, in_maps, list(range(NCORES)), **kw)


def kernel(inputs: np.ndarray, _return_raw=False, **run_kw) -> np.ndarray:
    assert inputs.shape == (B, NZ), inputs.shape
    zvec = np.ascontiguousarray(inputs, dtype=np.float32)

    # pack: leading 1.0 column + per-row [z..., 1.0 sentinel]
    zp = np.ones((B, PACKED + 1), np.float32)
    zp[:, 1 + _strict_slots] = zvec

    in_maps = [
        {"zp": np.ascontiguousarray(zp[c * B_CORE:(c + 1) * B_CORE])}
        for c in range(NCORES)
    ]
    res = _run(in_maps, **run_kw)

    lp = np.empty((B, PACKED), np.float32)
    for c in range(NCORES):
        lp[c * B_CORE:(c + 1) * B_CORE] = res.results[c]["lp"]

    out = np.zeros((B, N * N), np.float32)
    out[:, _out_pos] = lp
    out = out.reshape(B, N, N)
    if _return_raw:
        return out, res
    return out
